# revision 21
# baseline (speedup 1.0000x reference)
"""Trainium2 Bass kernel for nn_CSNN (spiking CNN, T=8 timesteps, B=32).

Data-parallel over batch: 8 cores x 4 samples each. Each core runs the full
network for its samples.

Network (per sample, T=8 steps, input replicated across T):
  conv1 5x5 s1 p1 (1->128)   + IF(vth=13515)   28x28 -> 26x26
  conv2 4x4 s2 p1 (128->128) + IF(71)          26x26 -> 13x13
  conv3 3x3 s1 p1 (128->192) + IF(93)          13x13 -> 13x13
  conv4 3x3 s1 p1 (192->128) + IF(144)         13x13 -> 13x13
  conv5 3x3 s2 p0 (128->128) + IF(79)          13x13 -> 6x6
  fc1 4608->2048 + IF(2475); fc2 2048->512 + IF(1357); fc3 512->10 + IF(388)
  output = mean of fc3 spikes over T  -> [B, 10]

Because the conv input is replicated across T, conv1 is computed once per
sample. Layer-1 IF with constant per-step input c never fires iff
T*max(c) < vth1; the kernel computes conv1, checks that bound on-device and
branches: if no layer-1 spikes can occur, every downstream activation is
exactly zero (biases are zero), so it writes zeros and skips the rest.
Otherwise it runs the full dense network (bf16 weights/spikes, fp32
accumulation, conv via shifted-window matmuls with channels on partitions).
"""

import numpy as np
import ml_dtypes

import bass_rust
import concourse.bacc as bacc
import concourse.bass as bass
import concourse.mybir as mybir
from concourse import tile
from concourse.bass_utils import run_bass_kernel_spmd

F32 = mybir.dt.float32
F32R = mybir.dt.float32r
BF16 = mybir.dt.bfloat16
I32 = mybir.dt.int32
AX = mybir.AxisListType
OP = mybir.AluOpType

T = 8
BC = 4          # batch per core
NCORES = 8
VTH = (13515.0, 71.0, 93.0, 144.0, 79.0, 2475.0, 1357.0, 388.0)
# conservative layer-1 no-spike bound: max(conv1) < vth/T (with rounding guard)
ZCHECK_THR = VTH[0] / T * (1.0 - 1e-3)

TAPS5 = [(dy, dx) for dy in range(5) for dx in range(5)]
TAPS4 = [(dy, dx) for dy in range(4) for dx in range(4)]
TAPS3 = [(dy, dx) for dy in range(3) for dx in range(3)]

_NC_CACHE = {}


def _if_step(nc, v_ap, c_ap, s_ap, vth, zz):
    """One IF-neuron step: v += c; s = (v >= vth); v = 0 where s."""
    if c_ap is not None:
        nc.vector.tensor_add(v_ap, v_ap, c_ap)
    nc.vector.tensor_scalar(s_ap, v_ap, vth, None, OP.is_ge)
    p = v_ap.shape[0]
    nc.vector.copy_predicated(v_ap, s_ap.bitcast(mybir.dt.uint16),
                              zz[0:p, 0:1].to_broadcast(v_ap.shape))


def _build(has_bias):
    nc = bacc.Bacc("TRN2", target_bir_lowering=False, debug=False,
                   enable_partition_id=False, monotonic_sem_count=0)
    hb1, hb2, hb3, hb4, hb5 = has_bias
    K1 = 26 if hb1 else 25

    x_d = nc.declare_dram_parameter("xw", [128, 128 + 2 * 338], BF16,
                                    isOutput=False)
    w2_d = nc.declare_dram_parameter("w2t", [16, 128, 128], BF16, isOutput=False)
    w3_d = nc.declare_dram_parameter("w3t", [9, 128, 192], BF16, isOutput=False)
    w4a_d = nc.declare_dram_parameter("w4at", [9, 128, 128], BF16, isOutput=False)
    w4b_d = nc.declare_dram_parameter("w4bt", [9, 64, 128], BF16, isOutput=False)
    w5_d = nc.declare_dram_parameter("w5t", [9, 128, 128], BF16, isOutput=False)
    fc1_d = nc.declare_dram_parameter("fc1p", [36, 128, 2048], BF16, isOutput=False)
    fc2_d = nc.declare_dram_parameter("fc2p", [16, 128, 512], BF16, isOutput=False)
    fc3_d = nc.declare_dram_parameter("fc3p", [4, 128, 10], BF16, isOutput=False)
    eye_d = nc.declare_dram_parameter("eye32", [32, 32], F32, isOutput=False)
    b_d = {}
    for i, (hb, co) in enumerate([(hb2, 128), (hb3, 192), (hb4, 128), (hb5, 128)]):
        if hb:
            b_d[i + 2] = nc.declare_dram_parameter(f"b{i + 2}r", [1, co], BF16,
                                                   isOutput=False)
    out_d = nc.declare_dram_parameter("out", [BC, 10], F32, isOutput=True)

    with tile.TileContext(nc) as tc:
        with tc.tile_pool(name="const", bufs=1) as const, \
             tc.tile_pool(name="work", bufs=1) as work:
            # ---------------- prologue: conv1 + no-spike check ----------------
            xcomb = const.tile([128, 128 + 2 * 338], BF16, name="xcomb")

            ALLE = [mybir.EngineType.PE, mybir.EngineType.Activation,
                    mybir.EngineType.DVE, mybir.EngineType.SP,
                    mybir.EngineType.Pool]
            tc.mark_branch_hint_location("zskip", hint="LikelyTaken",
                                         engines=ALLE)
            # conv1 row-packed: 4 concurrent K1-row groups (tile_position
            # row tiling), each group owns 676 of the 2704 output columns
            # split as 2x338 psum chunks.
            for k in range(4):
                eng = nc.sync if k % 2 == 0 else nc.scalar
                eng.dma_start(xcomb[32 * k:32 * (k + 1)], x_d[32 * k:32 * (k + 1)])
            m6 = work.tile([128, 8], F32, name="m6")
            nthr = work.tile([128, 1], F32, name="nthr")
            nc.vector.memset(nthr, -ZCHECK_THR)
            with tc.tile_pool(name="p0", bufs=1, space="PSUM") as p0:
                for i, (k, j) in enumerate((k, j) for k in range(4)
                                           for j in range(2)):
                    rows = slice(32 * k, 32 * k + K1)
                    c1p = p0.tile([128, 338], F32, name=f"c1p{i}")
                    nc.tensor.matmul(c1p, xcomb[rows, 0:128],
                                     xcomb[rows, 128 + 338 * j:
                                           128 + 338 * (j + 1)],
                                     start=True, stop=True,
                                     tile_position=(32 * k, 0))
                    if i >= 3:
                        nc.vector.reduce_max(m6[:, i:i + 1], c1p[:], axis=AX.X)
                    else:
                        # ACT relu-accumulate for the early chunks
                        # (sum(relu(c1-thr)) > 0 iff some c1 > thr)
                        nc.scalar.activation(c1p[:], c1p[:],
                                             mybir.ActivationFunctionType.Relu,
                                             bias=nthr[:],
                                             accum_out=m6[:, i:i + 1])
                mall = work.tile([1, 1], F32, name="mall")
                nc.gpsimd.reduce_max(mall, m6[:, 0:8], axis=AX.XYZWC)

            val = nc.values_load(mall[0:1, 0:1].bitcast(I32).to_broadcast((1, 1)))
            thr_bits = int(np.float32(ZCHECK_THR).view(np.int32))

            with tc.If(val < thr_bits, preferred_fallthrough_block=False,
                       label="zskip") as cmp:
                outz = work.tile([BC, 10], F32, name="outz")
                nc.vector.memset(outz, 0.0)
                nc.sync.dma_start(out_d[:], outz[:])

            with cmp.Else():
                _dense(nc, tc, const, work, xcomb, K1, out_d, w2_d,
                       w3_d, w4a_d, w4b_d, w5_d, fc1_d, fc2_d, fc3_d, eye_d,
                       b_d, has_bias)
    nc.compile()
    return nc


def _dense(nc, tc, const, work, xcomb, K1, out_d, w2_d, w3_d, w4a_d,
           w4b_d, w5_d, fc1_d, fc2_d, fc3_d, eye_d, b_d, has_bias):
    _, hb2, hb3, hb4, hb5 = has_bias

    # recompute conv1 into SBUF (prologue's psum banks are released)
    c1s = work.tile([128, BC * 26 * 26], F32, name="c1s")
    with tc.tile_pool(name="p1d", bufs=2, space="PSUM") as p1d:
        for k in range(4):
            rows = slice(32 * k, 32 * k + K1)
            for j in range(2):
                c1p = p1d.tile([128, 338], F32, name="c1pd")
                nc.tensor.matmul(c1p, xcomb[rows, 0:128],
                                 xcomb[rows, 128 + 338 * j:128 + 338 * (j + 1)],
                                 start=True, stop=True,
                                 tile_position=(32 * k, 0))
                off = 676 * k + 338 * j
                nc.scalar.copy(c1s[:, off:off + 338], c1p[:])

    w2s = const.tile([128, 16, 128], BF16, name="w2s")
    nc.sync.dma_start(w2s[:], w2_d[:].rearrange("k c o -> c k o"))
    w3s = const.tile([128, 9, 192], BF16, name="w3s")
    nc.sync.dma_start(w3s[:], w3_d[:].rearrange("k c o -> c k o"))
    w4as = const.tile([128, 9, 128], BF16, name="w4as")
    nc.sync.dma_start(w4as[:], w4a_d[:].rearrange("k c o -> c k o"))
    w4bs = const.tile([64, 9, 128], BF16, name="w4bs")
    nc.sync.dma_start(w4bs[:], w4b_d[:].rearrange("k c o -> c k o"))
    w5s = const.tile([128, 9, 128], BF16, name="w5s")
    nc.sync.dma_start(w5s[:], w5_d[:].rearrange("k c o -> c k o"))
    fc2s = const.tile([128, 16, 512], BF16, name="fc2s")
    nc.sync.dma_start(fc2s[:], fc2_d[:].rearrange("k c o -> c k o"))
    fc3s = const.tile([128, 4, 10], BF16, name="fc3s")
    nc.sync.dma_start(fc3s[:], fc3_d[:].rearrange("k c o -> c k o"))
    eyes = const.tile([32, 32], F32, name="eyes")
    nc.sync.dma_start(eyes[:], eye_d[:])
    zz = const.tile([128, 1], F32, name="zz")
    nc.vector.memset(zz, 0.0)
    bs = {}
    if any([hb2, hb3, hb4, hb5]):
        ones512 = const.tile([1, 512], BF16, name="ones512")
        nc.vector.memset(ones512, 1.0)
        for i in (2, 3, 4, 5):
            if i in b_d:
                co = b_d[i].shape[1]
                bs[i] = const.tile([1, co], BF16, name=f"b{i}s")
                nc.sync.dma_start(bs[i][:], b_d[i][:])

    # double-buffered padded spike planes (borders stay zero)
    s1w = [work.tile([128, BC, 28, 28], BF16, name=f"s1w{i}") for i in range(2)]
    s2w = [work.tile([128, BC, 15, 15], BF16, name=f"s2w{i}") for i in range(2)]
    s3aw = [work.tile([128, BC, 15, 15], BF16, name=f"s3aw{i}") for i in range(2)]
    s3bw = [work.tile([64, BC, 15, 15], BF16, name=f"s3bw{i}") for i in range(2)]
    s4w = [work.tile([128, BC, 15, 15], BF16, name=f"s4w{i}") for i in range(2)]
    for tl in s1w + s2w + s3aw + s3bw + s4w:
        nc.vector.memset(tl[:], 0.0)
    s5s = work.tile([128, T, BC, 36], BF16, name="s5s")

    v1 = work.tile([128, BC, 26, 26], F32, name="v1")
    v2 = work.tile([128, BC, 13, 13], F32, name="v2")
    v3a = work.tile([128, BC, 13, 13], F32, name="v3a")
    v3b = work.tile([64, BC, 13, 13], F32, name="v3b")
    v4 = work.tile([128, BC, 13, 13], F32, name="v4")
    v5 = work.tile([128, BC, 6, 6], F32, name="v5")
    for tl in (v1, v2, v3a, v3b, v4, v5):
        nc.vector.memset(tl[:], 0.0)

    c1v = c1s.rearrange("p (b i j) -> p b i j", b=BC, i=26)

    def bias_mm(ps, layer, n):
        if layer in bs:
            nc.tensor.matmul(ps, bs[layer][:], ones512[:, 0:n],
                             start=False, stop=False)

    with tc.tile_pool(name="pA", bufs=2, space="PSUM") as pA, \
         tc.tile_pool(name="pB", bufs=2, space="PSUM") as pB, \
         tc.tile_pool(name="pC", bufs=2, space="PSUM") as pC, \
         tc.tile_pool(name="pD", bufs=2, space="PSUM") as pD:
        for t in range(T):
            bf = t % 2
            # IF1 (input is c1 every step)
            _if_step(nc, v1[:], c1v, s1w[bf][:, :, 1:27, 1:27], VTH[0], zz)
            # conv2: 4x4 s2 p1, 26x26(+pad)->13x13
            for g in range(2):
                c2 = pA.tile([128, 338], F32, name="c2")
                for kk, (dy, dx) in enumerate(TAPS4):
                    rhs = s1w[bf][:, 2 * g:2 * g + 2, dy:dy + 25:2, dx:dx + 25:2]
                    nc.tensor.matmul(c2, w2s[:, kk, :], rhs,
                                     start=(kk == 0), stop=(kk == 15))
                bias_mm(c2, 2, 338)
                nc.vector.tensor_add(v2[:, 2 * g:2 * g + 2], v2[:, 2 * g:2 * g + 2],
                                     c2[:].rearrange("p (b i j) -> p b i j", b=2, i=13))
            _if_step(nc, v2[:], None, s2w[bf][:, :, 1:14, 1:14], VTH[1], zz)
            # conv3: 3x3 s1 p1, 13x13->13x13, 128->192
            for g in range(2):
                c3a = pB.tile([128, 338], F32, name="c3a")
                c3b = pC.tile([64, 338], F32, name="c3b")
                for kk, (dy, dx) in enumerate(TAPS3):
                    rhs = s2w[bf][:, 2 * g:2 * g + 2, dy:dy + 13, dx:dx + 13]
                    nc.tensor.matmul(c3a, w3s[:, kk, 0:128], rhs,
                                     start=(kk == 0), stop=(kk == 8))
                    nc.tensor.matmul(c3b, w3s[:, kk, 128:192], rhs,
                                     start=(kk == 0), stop=(kk == 8))
                bias_mm(c3a, 3, 338)
                nc.vector.tensor_add(v3a[:, 2 * g:2 * g + 2], v3a[:, 2 * g:2 * g + 2],
                                     c3a[:].rearrange("p (b i j) -> p b i j", b=2, i=13))
                nc.vector.tensor_add(v3b[:, 2 * g:2 * g + 2], v3b[:, 2 * g:2 * g + 2],
                                     c3b[:].rearrange("p (b i j) -> p b i j", b=2, i=13))
            _if_step(nc, v3a[:], None, s3aw[bf][:, :, 1:14, 1:14], VTH[2], zz)
            _if_step(nc, v3b[:], None, s3bw[bf][:, :, 1:14, 1:14], VTH[2], zz)
            # conv4: 3x3 s1 p1, 192->128 (contraction split 128+64)
            for g in range(2):
                c4 = pD.tile([128, 338], F32, name="c4")
                for kk, (dy, dx) in enumerate(TAPS3):
                    rhs = s3aw[bf][:, 2 * g:2 * g + 2, dy:dy + 13, dx:dx + 13]
                    nc.tensor.matmul(c4, w4as[:, kk, :], rhs,
                                     start=(kk == 0), stop=False)
                for kk, (dy, dx) in enumerate(TAPS3):
                    rhs = s3bw[bf][:, 2 * g:2 * g + 2, dy:dy + 13, dx:dx + 13]
                    nc.tensor.matmul(c4, w4bs[:, kk, :], rhs,
                                     start=False, stop=(kk == 8))
                bias_mm(c4, 4, 338)
                nc.vector.tensor_add(v4[:, 2 * g:2 * g + 2], v4[:, 2 * g:2 * g + 2],
                                     c4[:].rearrange("p (b i j) -> p b i j", b=2, i=13))
            _if_step(nc, v4[:], None, s4w[bf][:, :, 1:14, 1:14], VTH[3], zz)
            # conv5: 3x3 s2 p0, 13x13->6x6 (all 4 samples in one matmul)
            c5 = pA.tile([128, 338], F32, name="c2")   # shares pA slots
            c5v = c5[:, 0:144]
            for kk, (dy, dx) in enumerate(TAPS3):
                rhs = s4w[bf][:, :, dy + 1:dy + 12:2, dx + 1:dx + 12:2]
                nc.tensor.matmul(c5v, w5s[:, kk, :], rhs,
                                 start=(kk == 0), stop=(kk == 8))
            bias_mm(c5v, 5, 144)
            nc.vector.tensor_add(v5[:], v5[:],
                                 c5v.rearrange("p (b i j) -> p b i j", b=BC, i=6))
            nc.vector.tensor_scalar(s5s[:, t], v5[:], VTH[4], None, OP.is_ge)
            nc.vector.copy_predicated(v5[:], s5s[:, t].bitcast(mybir.dt.uint16),
                                      zz.to_broadcast((128, BC, 6, 6)))

    # ---------------- fc phase ----------------
    c6t = work.tile([128, 16, 32], F32, name="c6t")
    c6s = work.tile([32, 2048], F32, name="c6s")
    with tc.tile_pool(name="pF", bufs=1, space="PSUM") as pF, \
         tc.tile_pool(name="pT", bufs=2, space="PSUM") as pT, \
         tc.tile_pool(name="fcw", bufs=3) as fcw:
        c6 = [pF.tile([32, 512], F32, name=f"c6p{oc}") for oc in range(4)]
        for p in range(36):
            fw = fcw.tile([128, 2048], BF16, name="fc1w")
            nc.sync.dma_start(fw[:], fc1_d[p])
            lhs = s5s[:, :, :, p]
            for oc in range(4):
                nc.tensor.matmul(c6[oc], lhs, fw[:, 512 * oc:512 * (oc + 1)],
                                 start=(p == 0), stop=(p == 35))
        for oc in range(4):
            nc.scalar.copy(c6s[:, 512 * oc:512 * (oc + 1)], c6[oc][:])
            for j in range(4):
                tp = pT.tile([128, 32], F32, name="tp")
                nc.tensor.transpose(tp, c6s[:, 512 * oc + 128 * j:
                                            512 * oc + 128 * (j + 1)], eyes[:])
                nc.vector.tensor_copy(c6t[:, 4 * oc + j, :], tp[:])

        v6t = work.tile([128, 16, 4], F32, name="v6t")
        s6t = work.tile([128, 16, 32], BF16, name="s6t")
        nc.vector.memset(v6t[:], 0.0)
        for t in range(T):
            _if_step(nc, v6t[:], c6t[:, :, 4 * t:4 * t + 4],
                     s6t[:, :, 4 * t:4 * t + 4], VTH[5], zz)

        with tc.tile_pool(name="p7", bufs=1, space="PSUM") as p7:
            c7 = p7.tile([32, 512], F32, name="c7")
            for ik in range(16):
                nc.tensor.matmul(c7, s6t[:, ik, :], fc2s[:, ik, :],
                                 start=(ik == 0), stop=(ik == 15))
            c7s = work.tile([32, 512], F32, name="c7s")
            nc.scalar.copy(c7s[:], c7[:])
        c7t = work.tile([128, 4, 32], F32, name="c7t")
        for j in range(4):
            tp = pT.tile([128, 32], F32, name="tp")
            nc.tensor.transpose(tp, c7s[:, 128 * j:128 * (j + 1)], eyes[:])
            nc.vector.tensor_copy(c7t[:, j, :], tp[:])

        v7t = work.tile([128, 4, 4], F32, name="v7t")
        s7t = work.tile([128, 4, 32], BF16, name="s7t")
        nc.vector.memset(v7t[:], 0.0)
        for t in range(T):
            _if_step(nc, v7t[:], c7t[:, :, 4 * t:4 * t + 4],
                     s7t[:, :, 4 * t:4 * t + 4], VTH[6], zz)

        with tc.tile_pool(name="p8", bufs=1, space="PSUM") as p8:
            c8 = p8.tile([32, 10], F32, name="c8")
            for ik in range(4):
                nc.tensor.matmul(c8, s7t[:, ik, :], fc3s[:, ik, :],
                                 start=(ik == 0), stop=(ik == 3))
            c8s = work.tile([32, 10], F32, name="c8s")
            nc.scalar.copy(c8s[:], c8[:])
            tp8 = pT.tile([128, 32], F32, name="tp")
            nc.tensor.transpose(tp8[0:10, :], c8s[:], eyes[:])
            c8t = work.tile([10, 32], F32, name="c8t")
            nc.vector.tensor_copy(c8t[:], tp8[0:10, :])

        v8t = work.tile([10, 4], F32, name="v8t")
        s8t = work.tile([10, 32], BF16, name="s8t")
        nc.vector.memset(v8t[:], 0.0)
        for t in range(T):
            _if_step(nc, v8t[:], c8t[:, 4 * t:4 * t + 4],
                     s8t[:, 4 * t:4 * t + 4], VTH[7], zz)

        outT = work.tile([10, 4], F32, name="outT")
        nc.vector.reduce_sum(outT, s8t[:].rearrange("p (t b) -> p b t", b=BC),
                             axis=AX.X)
        nc.vector.tensor_scalar(outT, outT, 1.0 / T, None, OP.mult)
        nc.sync.dma_start(out_d[:].rearrange("b o -> o b"), outT[:])


def _prep_inputs(x, w1, b1, w2, b2, w3, b3, w4, b4, w5, b5, fc1, fc2, fc3):
    bf = ml_dtypes.bfloat16
    has_bias = tuple(bool(np.any(np.asarray(b) != 0))
                     for b in (b1, b2, b3, b4, b5))
    w1t = np.ascontiguousarray(
        np.asarray(w1, np.float32).reshape(128, 25).T)          # [25, 128]
    if has_bias[0]:
        w1t = np.concatenate([w1t, np.asarray(b1, np.float32)[None, :]], 0)
    shared = {
        "_w1t": w1t,
        "w2t": np.ascontiguousarray(
            np.asarray(w2, np.float32).transpose(2, 3, 1, 0)
            .reshape(16, 128, 128)).astype(bf),
        "w3t": np.ascontiguousarray(
            np.asarray(w3, np.float32).transpose(2, 3, 1, 0)
            .reshape(9, 128, 192)).astype(bf),
        "w5t": np.ascontiguousarray(
            np.asarray(w5, np.float32).transpose(2, 3, 1, 0)
            .reshape(9, 128, 128)).astype(bf),
        "fc1p": np.ascontiguousarray(
            np.asarray(fc1, np.float32).reshape(2048, 128, 36)
            .transpose(2, 1, 0)).astype(bf),
        "fc2p": np.ascontiguousarray(
            np.asarray(fc2, np.float32).reshape(512, 16, 128)
            .transpose(1, 2, 0)).astype(bf),
        "fc3p": np.ascontiguousarray(
            np.asarray(fc3, np.float32).reshape(10, 4, 128)
            .transpose(1, 2, 0)).astype(bf),
        "eye32": np.eye(32, dtype=np.float32),
    }
    w4t = np.ascontiguousarray(
        np.asarray(w4, np.float32).transpose(2, 3, 1, 0).reshape(9, 192, 128))
    shared["w4at"] = w4t[:, :128].astype(bf)
    shared["w4bt"] = np.ascontiguousarray(w4t[:, 128:]).astype(bf)
    for i, b in ((2, b2), (3, b3), (4, b4), (5, b5)):
        if has_bias[i - 1]:
            shared[f"b{i}r"] = np.asarray(b, np.float32)[None, :].astype(bf)
    return has_bias, shared


def _run(inputs, trace=False):
    x = np.asarray(inputs["x"], np.float32)
    has_bias, shared = _prep_inputs(**inputs)
    if has_bias not in _NC_CACHE:
        _NC_CACHE[has_bias] = _build(has_bias)
    nc = _NC_CACHE[has_bias]
    w1t = shared.pop("_w1t")
    xpad = np.pad(x[:, 0], ((0, 0), (1, 1), (1, 1)))          # [B, 30, 30]
    win = np.lib.stride_tricks.sliding_window_view(
        xpad, (26, 26), axis=(1, 2))                          # [B, 5, 5, 26, 26]
    in_maps = []
    for c in range(NCORES):
        m = dict(shared)
        xw = np.ascontiguousarray(
            win[c * BC:(c + 1) * BC].transpose(1, 2, 0, 3, 4)
            .reshape(25, BC * 26 * 26))
        if has_bias[0]:
            xw = np.concatenate([xw, np.ones((1, xw.shape[1]), np.float32)], 0)
        k1 = xw.shape[0]
        comb = np.zeros((128, 128 + 2 * 338), ml_dtypes.bfloat16)
        for k in range(4):
            comb[32 * k:32 * k + k1, 0:128] = w1t
            comb[32 * k:32 * k + k1, 128:] = xw[:, 676 * k:676 * (k + 1)]
        m["xw"] = comb
        in_maps.append(m)
    res = run_bass_kernel_spmd(nc, in_maps, list(range(NCORES)), trace=trace)
    out = np.concatenate([res.results[c]["out"] for c in range(NCORES)], axis=0)
    return out.astype(np.float32), res


def kernel(**inputs):
    out, _ = _run(inputs)
    return out
